# revision 1
# baseline (speedup 1.0000x reference)
"""Trainium2 Bass kernel for nn_AttentionToVec (B=8, N=4096, E=1024, H=16, D=64).

Strategy: data-parallel over batch (1 batch element per NeuronCore) for the
attention part; tensor-parallel over the MLP hidden dim (4096/8=512 per core)
with an AllGather of the per-core sampled vectors and a ReduceScatter of the
partial MLP outputs (which lands exactly each core's own output row).

Algebraic restructuring (host does weight-only folding):
  - att logits = x @ w_att where w_att[e,h] = sum_d W_k[e, h*D+d] * query[h,d]
    (the k-projection bias cancels inside softmax over n).
  - y[h,:] = sum_n softmax_att[n,h] * x[n,:]  (deferred 1/Z normalization)
  - sampled[h,d] = (y[h,:] @ W_v[:, h*D+d]) + b_v[h*D+d]   (sum_n att = 1)
"""

import numpy as np

B = 8
N = 4096
E = 1024
H = 16
D = 64
HID = 4096
NCORES = 8
HID_C = HID // NCORES

# Matmul operand dtype knobs per stage: "f32" | "f32r" | "bf16"
DT_ATT = "bf16"   # step 1: attT = w_attT @ xT
DT_Y = "f32r"     # step 3: y = exp_attT @ x (+ Z)
DT_V = "f32r"     # step 4: y @ W_v
DT_MLP = "f32r"   # MLP matmuls

_CACHE = {}


def _np_dt(knob):
    if knob == "bf16":
        import ml_dtypes

        return np.dtype(ml_dtypes.bfloat16)
    return np.dtype(np.float32)


def _build():
    import concourse.bacc as bacc
    import concourse.mybir as mybir
    from concourse import tile
    from concourse.masks import make_identity

    f32 = mybir.dt.float32
    bf16 = mybir.dt.bfloat16
    f32r = mybir.dt.float32r
    Act = mybir.ActivationFunctionType
    Alu = mybir.AluOpType

    def store_dt(knob):
        if knob == "bf16":
            return bf16
        if knob == "f32r":
            return f32r
        return f32

    def mm_ap(ap, knob):
        # tiles are already declared in the matmul dtype
        return ap

    nc = bacc.Bacc(None, target_bir_lowering=False, debug=True, num_devices=NCORES)

    dt_att = store_dt(DT_ATT)
    dt_y = store_dt(DT_Y)
    dt_v = store_dt(DT_V)
    dt_mlp = store_dt(DT_MLP)

    xT = nc.dram_tensor("xT", [E, N], dt_att, kind="ExternalInput")
    x = nc.dram_tensor("x", [N, E], dt_y, kind="ExternalInput")
    watt = nc.dram_tensor("watt", [E, H], dt_att, kind="ExternalInput")
    amask = nc.dram_tensor("amask", [H, N], f32, kind="ExternalInput")
    Wv = nc.dram_tensor("Wv", [E, E], dt_v, kind="ExternalInput")
    bvb = nc.dram_tensor("bvb", [H, E], f32, kind="ExternalInput")
    W1c = nc.dram_tensor("W1c", [E, HID_C], dt_mlp, kind="ExternalInput")
    b1c = nc.dram_tensor("b1c", [NCORES, HID_C], f32, kind="ExternalInput")
    W2c = nc.dram_tensor("W2c", [HID_C, E], dt_mlp, kind="ExternalInput")
    b2r8 = nc.dram_tensor("b2r8", [NCORES, E], f32, kind="ExternalInput")
    ones2 = nc.dram_tensor("ones2", [128, 2], dt_y, kind="ExternalInput")
    out = nc.dram_tensor("out", [1, E], f32, kind="ExternalOutput")

    with tile.TileContext(nc) as tc:
        with (
            tc.tile_pool(name="consts", bufs=1) as consts,
            tc.tile_pool(name="xtp", bufs=2) as xtp,
            tc.tile_pool(name="xp", bufs=4) as xp,
            tc.tile_pool(name="wvp", bufs=2) as wvp,
            tc.tile_pool(name="wmlp", bufs=1) as wmlp,
            tc.tile_pool(name="work", bufs=1) as work,
            tc.tile_pool(name="dramp", bufs=1, space="DRAM") as dramp,
        ):
            identity = consts.tile([128, 128], f32)
            make_identity(nc, identity[:])
            ones_col = consts.tile([128, 2], dt_y)
            nc.sync.dma_start(out=ones_col[:], in_=ones2[:, :])

            watt_s = consts.tile([128, 8, H], dt_att)
            nc.sync.dma_start(
                out=watt_s[:], in_=watt.ap().rearrange("(c p) h -> p c h", p=128)
            )
            amask_s = consts.tile([H, N], f32)
            nc.sync.dma_start(out=amask_s[:], in_=amask[:, :])
            bvb_s = consts.tile([H, E], f32)
            nc.sync.dma_start(out=bvb_s[:], in_=bvb[:, :])
            b1_s = consts.tile([NCORES, HID_C], f32)
            nc.sync.dma_start(out=b1_s[:], in_=b1c[:, :])
            b28_s = consts.tile([NCORES, E], f32)
            nc.sync.dma_start(out=b28_s[:], in_=b2r8[:, :])

            # ---- Phase A: attT[16, N] = w_att^T @ x^T (accumulate over e) ----
            psA_cm = tc.tile_pool(name="psA", bufs=1, space="PSUM")
            psA = psA_cm.__enter__()
            attT = psA.tile([H, N], f32)
            for c in range(8):
                xt = xtp.tile([128, N], dt_att, tag="xT")
                nc.sync.dma_start(out=xt[:], in_=xT[128 * c : 128 * (c + 1), :])
                for j in range(8):
                    nc.tensor.matmul(
                        attT[:, 512 * j : 512 * (j + 1)],
                        mm_ap(watt_s[:, c, :], DT_ATT),
                        mm_ap(xt[:, 512 * j : 512 * (j + 1)], DT_ATT),
                        start=(c == 0),
                        stop=(c == 7),
                    )

            # masked logits -> SBUF
            attm = work.tile([H, N], f32)
            for j in range(8):
                sl = slice(512 * j, 512 * (j + 1))
                nc.vector.tensor_add(attm[:, sl], attT[:, sl], amask_s[:, sl])
            psA_cm.__exit__(None, None, None)
            psTr_cm = tc.tile_pool(name="psTr", bufs=4, space="PSUM")
            psTr = psTr_cm.__enter__()
            psB_cm = tc.tile_pool(name="psB", bufs=1, space="PSUM")
            psB = psB_cm.__enter__()

            # ---- Phase A2 + B fused: per n-tile transpose+exp, then y/Z accum ----
            att_n = work.tile([128, 32 * H], dt_y)
            for t in range(32):
                tr = psTr.tile([128, H], f32, tag="tr")
                nc.tensor.transpose(
                    tr[:], attm[:, 128 * t : 128 * (t + 1)], identity[:H, :H]
                )
                nc.scalar.activation(att_n[:, H * t : H * (t + 1)], tr[:], Act.Exp)

            y_ps = psB.tile([H, E], f32, tag="acc")
            z_ps = psB.tile([H, 2], f32, tag="accz")
            xr = x.ap().rearrange("(tt u p) e -> tt p u e", u=2, p=128)
            for tt in range(16):
                xt2 = xp.tile([128, 2, E], dt_y, tag="x")
                nc.sync.dma_start(out=xt2[:], in_=xr[tt])
                for u in range(2):
                    t = 2 * tt + u
                    lhs = mm_ap(att_n[:, H * t : H * (t + 1)], DT_Y)
                    nc.tensor.matmul(
                        y_ps[:, 0:512],
                        lhs,
                        mm_ap(xt2[:, u, 0:512], DT_Y),
                        start=(t == 0),
                        stop=(t == 31),
                    )
                    nc.tensor.matmul(
                        y_ps[:, 512:1024],
                        lhs,
                        mm_ap(xt2[:, u, 512:1024], DT_Y),
                        start=(t == 0),
                        stop=(t == 31),
                    )
                    nc.tensor.matmul(
                        z_ps[:],
                        lhs,
                        mm_ap(ones_col[:], DT_Y),
                        start=(t == 0),
                        stop=(t == 31),
                    )

            rz = work.tile([H, 1], f32)
            nc.vector.reciprocal(rz[:], z_ps[:, 0:1])
            y_s = work.tile([H, E], f32)
            nc.vector.tensor_scalar_mul(y_s[:], y_ps[:], rz[:])

            # ---- Phase C: sampled = diag_blocks(y @ Wv) + bv ----
            yT = work.tile([128, 8 * H], dt_v)
            for j in range(8):
                tr2 = psTr.tile([128, H], f32, tag="tr")
                nc.tensor.transpose(
                    tr2[:], y_s[:, 128 * j : 128 * (j + 1)], identity[:H, :H]
                )
                nc.vector.tensor_copy(yT[:, H * j : H * (j + 1)], tr2[:])

            sf_ps = psB.tile([H, E], f32, tag="acc")
            for c in range(8):
                wv_t = wvp.tile([128, E], dt_v, tag="wv")
                nc.sync.dma_start(out=wv_t[:], in_=Wv[128 * c : 128 * (c + 1), :])
                for j in range(2):
                    nc.tensor.matmul(
                        sf_ps[:, 512 * j : 512 * (j + 1)],
                        mm_ap(yT[:, H * c : H * (c + 1)], DT_V),
                        mm_ap(wv_t[:, 512 * j : 512 * (j + 1)], DT_V),
                        start=(c == 0),
                        stop=(c == 7),
                    )

            # sampled[h, d] = sf[h, h*D + d] (bias already folded in).
            # Bounce sf through DRAM with padded rows; the AllGather input is a
            # skewed strided view that picks exactly the diagonal blocks.
            sf_s = work.tile([H, E], f32)
            nc.vector.tensor_add(sf_s[:], sf_ps[:], bvb_s[:])
            sf_d = dramp.tile([H, E + D], f32)
            nc.sync.dma_start(out=sf_d[:, :E], in_=sf_s[:])
            import concourse.bass as bass_mod

            sfd_ap = sf_d[:]
            diag_view = bass_mod.AP(
                tensor=sfd_ap.tensor, offset=0, ap=[[E + 2 * D, H], [1, D]]
            )
            s_loc = work.tile([H, D], f32)
            nc.sync.dma_start(out=s_loc[:], in_=diag_view)
            s_dram = dramp.tile([1, E], f32)
            nc.sync.dma_start(
                out=s_dram[:].rearrange("o (h d) -> (o h) d", h=H), in_=s_loc[:]
            )

            # ---- Phase D: AllGather sampled vectors ----
            S_all = dramp.tile([NCORES, E], f32, addr_space="Shared")
            nc.gpsimd.collective_compute(
                "AllGather",
                Alu.bypass,
                replica_groups=[list(range(NCORES))],
                ins=[s_dram[:].opt()],
                outs=[S_all[:].opt()],
            )

            S_s = work.tile([NCORES, E], f32)
            nc.sync.dma_start(out=S_s[:], in_=S_all[:])
            ST = work.tile([128, 8 * NCORES], dt_mlp)
            for j in range(8):
                tr3 = psTr.tile([128, H], f32, tag="tr")
                nc.tensor.transpose(
                    tr3[:, :NCORES],
                    S_s[:, 128 * j : 128 * (j + 1)],
                    identity[:NCORES, :NCORES],
                )
                nc.vector.tensor_copy(
                    ST[:, NCORES * j : NCORES * (j + 1)], tr3[:, :NCORES]
                )

            # ---- Phase E: MLP (tensor-parallel over hidden slice) ----
            w1_s = wmlp.tile([128, 8, HID_C], dt_mlp, tag="w1")
            nc.sync.dma_start(
                out=w1_s[:], in_=W1c.ap().rearrange("(c p) m -> p c m", p=128)
            )
            h1_ps = psB.tile([NCORES, HID_C], f32, tag="accz")
            for c in range(8):
                nc.tensor.matmul(
                    h1_ps[:],
                    mm_ap(ST[:, NCORES * c : NCORES * (c + 1)], DT_MLP),
                    mm_ap(w1_s[:, c, :], DT_MLP),
                    start=(c == 0),
                    stop=(c == 7),
                )

            # z = h1 + b1 ; gelu (tanh approx, matches jax.nn.gelu default)
            z_s = work.tile([NCORES, HID_C], f32)
            nc.vector.tensor_add(z_s[:], h1_ps[:], b1_s[:])
            sq = work.tile([NCORES, HID_C], f32, tag="ga")
            nc.scalar.activation(sq[:], z_s[:], Act.Square)
            cube = work.tile([NCORES, HID_C], f32, tag="gb")
            nc.vector.tensor_mul(cube[:], sq[:], z_s[:])
            uu = work.tile([NCORES, HID_C], f32, tag="ga")
            nc.vector.scalar_tensor_tensor(
                uu[:], cube[:], 0.044715, z_s[:], Alu.mult, Alu.add
            )
            th = work.tile([NCORES, HID_C], f32, tag="gb")
            nc.scalar.activation(th[:], uu[:], Act.Tanh, scale=0.7978845608028654)
            hh2 = work.tile([NCORES, HID_C], f32, tag="ga")
            nc.vector.scalar_tensor_tensor(
                hh2[:], th[:], 1.0, z_s[:], Alu.add, Alu.mult
            )
            nc.vector.tensor_scalar_mul(hh2[:], hh2[:], 0.5)

            hT = work.tile([128, 4 * NCORES], dt_mlp)
            for j in range(4):
                tr4 = psTr.tile([128, H], f32, tag="tr")
                nc.tensor.transpose(
                    tr4[:, :NCORES],
                    hh2[:, 128 * j : 128 * (j + 1)],
                    identity[:NCORES, :NCORES],
                )
                nc.vector.tensor_copy(
                    hT[:, NCORES * j : NCORES * (j + 1)], tr4[:, :NCORES]
                )

            w2_s = wmlp.tile([128, 4, E], dt_mlp, tag="w2")
            nc.sync.dma_start(
                out=w2_s[:], in_=W2c.ap().rearrange("(c p) e -> p c e", p=128)
            )
            p2_ps = psB.tile([NCORES, E], f32, tag="acc")
            for c in range(4):
                for j in range(2):
                    nc.tensor.matmul(
                        p2_ps[:, 512 * j : 512 * (j + 1)],
                        mm_ap(hT[:, NCORES * c : NCORES * (c + 1)], DT_MLP),
                        mm_ap(w2_s[:, c, 512 * j : 512 * (j + 1)], DT_MLP),
                        start=(c == 0),
                        stop=(c == 3),
                    )

            sb8 = work.tile([NCORES, E], f32)
            nc.vector.scalar_tensor_tensor(
                sb8[:], S_s[:], 0.125, b28_s[:], Alu.mult, Alu.add
            )
            mlp_s = work.tile([NCORES, E], f32)
            nc.vector.tensor_add(mlp_s[:], p2_ps[:], sb8[:])
            mlp_d = dramp.tile([NCORES, E], f32)
            nc.sync.dma_start(out=mlp_d[:], in_=mlp_s[:])

            # ---- Phase F: ReduceScatter -> this core's output row ----
            mlp_row = dramp.tile([1, E], f32)
            nc.gpsimd.collective_compute(
                "ReduceScatter",
                Alu.add,
                replica_groups=[list(range(NCORES))],
                ins=[mlp_d[:].opt()],
                outs=[mlp_row[:].opt()],
            )

            m_row = work.tile([1, E], f32)
            nc.sync.dma_start(out=m_row[:], in_=mlp_row[:])
            nc.sync.dma_start(out=out[:, :], in_=m_row[:])
            psB_cm.__exit__(None, None, None)
            psTr_cm.__exit__(None, None, None)

    return nc


def get_nc():
    if "nc" not in _CACHE:
        nc = _build()
        nc.finalize()
        _CACHE["nc"] = nc
    return _CACHE["nc"]


def build_in_maps(x, mask, W_kv, b_kv, query, W1, b1, W2, b2):
    """Host-side shard prep. Weight-only algebra + layout transforms."""
    x = np.asarray(x, np.float32)
    mask = np.asarray(mask)
    W_kv = np.asarray(W_kv, np.float32)
    b_kv = np.asarray(b_kv, np.float32)
    query = np.asarray(query, np.float32)
    W1 = np.asarray(W1, np.float32)
    b1 = np.asarray(b1, np.float32)
    W2 = np.asarray(W2, np.float32)
    b2 = np.asarray(b2, np.float32)

    W_k = W_kv[:, :E]
    W_v = W_kv[:, E:]
    # fold the per-head query into the k-projection: [E, H]
    w_att = np.einsum("ehd,hd->eh", W_k.reshape(E, H, D), query).astype(np.float32)
    bv_b = np.ascontiguousarray(np.broadcast_to(b_kv[None, E:], (H, E)).astype(np.float32))

    addmask = np.where(mask[:, :, 0], np.float32(-1e30), np.float32(0.0))  # [B, N]

    dt_att = _np_dt(DT_ATT)
    dt_y = _np_dt(DT_Y)
    dt_v = _np_dt(DT_V)
    dt_mlp = _np_dt(DT_MLP)

    Wv_c = np.ascontiguousarray(W_v.astype(dt_v))
    watt_c = np.ascontiguousarray(w_att.astype(dt_att))
    b2r8 = np.ascontiguousarray(np.broadcast_to(b2[None, :] / 8.0, (NCORES, E)).astype(np.float32))

    in_maps = []
    for c in range(NCORES):
        hs = slice(HID_C * c, HID_C * (c + 1))
        in_maps.append(
            {
                "xT": np.ascontiguousarray(x[c].T.astype(dt_att)),
                "x": np.ascontiguousarray(x[c].astype(dt_y)),
                "watt": watt_c,
                "amask": np.ascontiguousarray(
                    np.broadcast_to(addmask[c][None, :], (H, N))
                ),
                "Wv": Wv_c,
                "bvb": bv_b,
                "W1c": np.ascontiguousarray(W1[:, hs].astype(dt_mlp)),
                "b1c": np.ascontiguousarray(np.broadcast_to(b1[hs][None, :], (NCORES, HID_C))),
                "W2c": np.ascontiguousarray(W2[hs, :].astype(dt_mlp)),
                "b2r8": b2r8,
                "ones2": np.ones((128, 2), dtype=dt_y),
            }
        )
    return in_maps


def kernel(**inputs):
    from concourse.bass_utils import run_bass_kernel_spmd

    in_maps = build_in_maps(**inputs)
    nc = get_nc()
    res = run_bass_kernel_spmd(nc, in_maps, list(range(NCORES)), trace=False)
    return np.stack([res.results[c]["out"][0] for c in range(NCORES)]).astype(
        np.float32
    )



# revision 8
# speedup vs baseline: 1.6494x; 1.6494x over previous
"""Trainium2 Bass kernel for nn_AttentionToVec (B=8, N=4096, E=1024, H=16, D=64).

Strategy: pure data-parallel over batch (1 batch element per NeuronCore), NO
collectives.  Each core computes its own row's full MLP with the complete
W1/W2 (profiling showed the AllGather/ReduceScatter + cc-barrier of the
tensor-parallel MLP cost ~100us, far more than the extra weight traffic).

Dtypes (validated vs reference on host, rel-err ~1.3e-3 vs 2e-2 budget):
  - xT (phase A moving operand): fp8 e4m3, mixed with fp16 stationary w_att
  - x (phase B), Wv, W1, W2, p=exp(att), yT, s, h: fp16
  - all matmul accumulation fp32 in PSUM

Algebraic restructuring (host does weight-only folding):
  - att logits = x @ w_att where w_att[e,h] = sum_d W_k[e, h*D+d] * query[h,d]
    (the k-projection bias cancels inside softmax over n).
  - y[h,:] = sum_n exp_att[n,h] * x[n,:]  (deferred 1/Z normalization)
  - sampled[e] = (y[h(e),:] @ W_v[:, e]) + b_v[e],  h(e)=e//D.  Phase C
    computes ONLY the needed diagonal blocks, directly transposed:
    sfT_j[m, i] = sf[2j+i, 128j+m] so s[128j+m] = sfT_j[m, m//64].
  - MLP per-core on its own row, hidden laid out as zT[p, q] = z[128q+p]
    so gelu runs across all 128 partitions.
"""

import numpy as np

B = 8
N = 4096
E = 1024
H = 16
D = 64
HID = 4096
NCORES = 8

_CACHE = {}


def _build():
    import concourse.bacc as bacc
    import concourse.mybir as mybir
    from concourse import tile
    from concourse.masks import make_identity

    f32 = mybir.dt.float32
    f16 = mybir.dt.float16
    f8 = mybir.dt.float8e4
    Act = mybir.ActivationFunctionType
    Alu = mybir.AluOpType

    nc = bacc.Bacc(None, target_bir_lowering=False, debug=True, num_devices=NCORES)

    xT8 = nc.dram_tensor("xT8", [E, N], f8, kind="ExternalInput")
    x16 = nc.dram_tensor("x16", [N, E], f16, kind="ExternalInput")
    watt = nc.dram_tensor("watt", [E, H], f16, kind="ExternalInput")
    maskn = nc.dram_tensor("maskn", [128, 32], f32, kind="ExternalInput")
    wv = nc.dram_tensor("wv", [E, E], f16, kind="ExternalInput")
    bvT = nc.dram_tensor("bvT", [128, 8], f32, kind="ExternalInput")
    w1 = nc.dram_tensor("w1", [E, HID], f16, kind="ExternalInput")
    b1T = nc.dram_tensor("b1T", [128, 32], f32, kind="ExternalInput")
    w2 = nc.dram_tensor("w2", [HID, E], f16, kind="ExternalInput")
    b2T = nc.dram_tensor("b2T", [128, 8], f32, kind="ExternalInput")
    out = nc.dram_tensor("out", [1, E], f32, kind="ExternalOutput")

    with tile.TileContext(nc) as tc:
        with (
            tc.tile_pool(name="consts", bufs=1) as consts,
            tc.tile_pool(name="xtp", bufs=2) as xtp,
            tc.tile_pool(name="xp", bufs=2) as xp,
            tc.tile_pool(name="wvp", bufs=1) as wvp,
            tc.tile_pool(name="wmlp", bufs=1) as wmlp,
            tc.tile_pool(name="work", bufs=1) as work,
        ):
            identity = consts.tile([H, H], f16)
            make_identity(nc, identity[:])
            ones_s = consts.tile([128, 2], f16)
            nc.vector.memset(ones_s[:], 1.0)

            watt_s = consts.tile([128, 8, H], f16)
            nc.sync.dma_start(
                out=watt_s[:], in_=watt.ap().rearrange("(c p) h -> p c h", p=128)
            )
            maskn_s = consts.tile([128, 32], f32)
            nc.sync.dma_start(out=maskn_s[:], in_=maskn[:, :])
            bvT_s = consts.tile([128, 8], f32)
            nc.sync.dma_start(out=bvT_s[:], in_=bvT[:, :])
            b1T_s = consts.tile([128, 32], f32)
            nc.sync.dma_start(out=b1T_s[:], in_=b1T[:, :])
            b2T_s = consts.tile([128, 8], f32)
            nc.sync.dma_start(out=b2T_s[:], in_=b2T[:, :])

            # ---- Phase A: attT[16, N] = w_att^T @ x^T (accumulate over e) ----
            psA_cm = tc.tile_pool(name="psA", bufs=1, space="PSUM")
            psA = psA_cm.__enter__()
            attT = psA.tile([H, N], f32)
            for cc in range(4):
                xt = xtp.tile([128, 2, N], f8, tag="xT")
                nc.sync.dma_start(
                    out=xt[:], in_=xT8[256 * cc : 256 * (cc + 1), :].rearrange(
                        "(u p) n -> p u n", p=128
                    )
                )
                for u in range(2):
                    c = 2 * cc + u
                    for j in range(8):
                        nc.tensor.matmul(
                            attT[:, 512 * j : 512 * (j + 1)],
                            watt_s[:, c, :],
                            xt[:, u, 512 * j : 512 * (j + 1)],
                            start=(c == 0),
                            stop=(c == 7),
                        )

            # logits -> SBUF fp16 (mask is applied later as the exp bias)
            attm = work.tile([H, N], f16)
            for j in range(8):
                sl = slice(512 * j, 512 * (j + 1))
                nc.vector.tensor_copy(attm[:, sl], attT[:, sl])
            psA_cm.__exit__(None, None, None)

            psTr_cm = tc.tile_pool(name="psTr", bufs=4, space="PSUM")
            psTr = psTr_cm.__enter__()
            psB_cm = tc.tile_pool(name="psB", bufs=1, space="PSUM")
            psB = psB_cm.__enter__()

            # ---- Phase A2: per n-tile transpose + exp(logit + mask) -> fp16 ----
            att_n = work.tile([128, 32 * H], f16)
            for t in range(32):
                tr = psTr.tile([128, H], f16, tag="tr")
                nc.tensor.transpose(
                    tr[:], attm[:, 128 * t : 128 * (t + 1)], identity[:, :]
                )
                nc.scalar.activation(
                    att_n[:, H * t : H * (t + 1)],
                    tr[:],
                    Act.Exp,
                    bias=maskn_s[:, t : t + 1],
                )

            # ---- Phase B: y[h,:] = sum_n p[n,h] x[n,:],  z[h] = sum_n p[n,h] ----
            y_ps = psB.tile([H, E], f32, tag="acc")
            z_ps = psB.tile([H, 2], f32, tag="accz")
            xr = x16.ap().rearrange("(tt u p) e -> tt p u e", u=4, p=128)
            for tt in range(8):
                xt2 = xp.tile([128, 4, E], f16, tag="x")
                nc.sync.dma_start(out=xt2[:], in_=xr[tt])
                for u in range(4):
                    t = 4 * tt + u
                    lhs = att_n[:, H * t : H * (t + 1)]
                    nc.tensor.matmul(
                        y_ps[:, 0:512],
                        lhs,
                        xt2[:, u, 0:512],
                        start=(t == 0),
                        stop=(t == 31),
                    )
                    nc.tensor.matmul(
                        y_ps[:, 512:1024],
                        lhs,
                        xt2[:, u, 512:1024],
                        start=(t == 0),
                        stop=(t == 31),
                    )
                    nc.tensor.matmul(
                        z_ps[:],
                        lhs,
                        ones_s[:],
                        start=(t == 0),
                        stop=(t == 31),
                    )

            # weight streams for the tail phases (issued here so they drain
            # during phases A/B without delaying the xT/x streams)
            wv_s = wvp.tile([128, 8, 8, 128], f16)
            nc.sync.dma_start(
                out=wv_s[:],
                in_=wv.ap().rearrange("(c p) (j m) -> p c j m", p=128, m=128),
            )
            w1_s = wmlp.tile([128, 8, 32, 128], f16, tag="w1")
            nc.sync.dma_start(
                out=w1_s[:],
                in_=w1.ap().rearrange("(c p) (q m) -> p c q m", p=128, m=128),
            )
            w2_s = wmlp.tile([128, 32, 8, 128], f16, tag="w2")
            nc.sync.dma_start(
                out=w2_s[:],
                in_=w2.ap().rearrange("(q p) (r m) -> p q r m", p=128, m=128),
            )

            # normalize: y = y / z
            rz = work.tile([H, 1], f32)
            nc.vector.reciprocal(rz[:], z_ps[:, 0:1])
            y_s = work.tile([H, E], f16)
            nc.vector.tensor_scalar_mul(y_s[:], y_ps[:], rz[:])
            psB_cm.__exit__(None, None, None)

            # yT[e, h] chunks (fp16) for phase C
            yT = work.tile([128, 8 * H], f16)
            for j in range(8):
                tr2 = psTr.tile([128, H], f16, tag="tr")
                nc.tensor.transpose(
                    tr2[:], y_s[:, 128 * j : 128 * (j + 1)], identity[:, :]
                )
                nc.vector.tensor_copy(yT[:, H * j : H * (j + 1)], tr2[:])

            # ---- Phase C: diagonal blocks of sf = y @ Wv, directly transposed.
            # sfT_j[m, i] = sf[2j+i, 128j+m]; s[128j+m] = sfT_j[m, m//64].
            psC_cm = tc.tile_pool(name="psC", bufs=2, space="PSUM")
            psC = psC_cm.__enter__()
            s_f = work.tile([128, 8], f32)
            for j in range(8):
                sfT = psC.tile([128, 2], f32, tag="sf")
                for c in range(8):
                    nc.tensor.matmul(
                        sfT[:],
                        wv_s[:, c, j, :],
                        yT[:, 16 * c + 2 * j : 16 * c + 2 * j + 2],
                        start=(c == 0),
                        stop=(c == 7),
                    )
                nc.vector.tensor_copy(s_f[0:64, j : j + 1], sfT[0:64, 0:1])
                nc.vector.tensor_copy(s_f[64:128, j : j + 1], sfT[64:128, 1:2])
            psC_cm.__exit__(None, None, None)

            nc.vector.tensor_add(s_f[:], s_f[:], bvT_s[:])
            s16 = work.tile([128, 8], f16)
            nc.vector.tensor_copy(s16[:], s_f[:])

            # ---- Phase E: full MLP for this core's row ----
            psM_cm = tc.tile_pool(name="psM", bufs=1, space="PSUM")
            psM = psM_cm.__enter__()
            zT_ps = psM.tile([128, 32], f32, tag="z")
            for q in range(32):
                for c in range(8):
                    nc.tensor.matmul(
                        zT_ps[:, q : q + 1],
                        w1_s[:, c, q, :],
                        s16[:, c : c + 1],
                        start=(c == 0),
                        stop=(c == 7),
                    )

            # gelu (tanh approx, matches jax.nn.gelu default)
            z_s = work.tile([128, 32], f32, tag="zs")
            nc.vector.tensor_add(z_s[:], zT_ps[:], b1T_s[:])
            sq = work.tile([128, 32], f32, tag="ga")
            nc.scalar.activation(sq[:], z_s[:], Act.Square)
            cube = work.tile([128, 32], f32, tag="gb")
            nc.vector.tensor_mul(cube[:], sq[:], z_s[:])
            uu = work.tile([128, 32], f32, tag="ga")
            nc.vector.scalar_tensor_tensor(
                uu[:], cube[:], 0.044715, z_s[:], Alu.mult, Alu.add
            )
            th = work.tile([128, 32], f32, tag="gb")
            nc.scalar.activation(th[:], uu[:], Act.Tanh, scale=0.7978845608028654)
            hh = work.tile([128, 32], f32, tag="ga")
            nc.vector.scalar_tensor_tensor(
                hh[:], th[:], 1.0, z_s[:], Alu.add, Alu.mult
            )
            h16 = work.tile([128, 32], f16, tag="h16")
            nc.vector.tensor_scalar_mul(h16[:], hh[:], 0.5)

            oT_ps = psM.tile([128, 8], f32, tag="o")
            for r in range(8):
                for q in range(32):
                    nc.tensor.matmul(
                        oT_ps[:, r : r + 1],
                        w2_s[:, q, r, :],
                        h16[:, q : q + 1],
                        start=(q == 0),
                        stop=(q == 31),
                    )

            of = work.tile([128, 8], f32, tag="of")
            nc.vector.tensor_add(of[:], oT_ps[:], b2T_s[:])
            nc.vector.tensor_add(of[:], of[:], s_f[:])
            nc.sync.dma_start(
                out=out.ap().rearrange("o (j p) -> p (o j)", p=128), in_=of[:]
            )
            psM_cm.__exit__(None, None, None)
            psTr_cm.__exit__(None, None, None)

    return nc


def get_nc():
    if "nc" not in _CACHE:
        nc = _build()
        nc.finalize()
        _CACHE["nc"] = nc
    return _CACHE["nc"]


def build_in_maps(x, mask, W_kv, b_kv, query, W1, b1, W2, b2):
    """Host-side shard prep. Weight-only algebra + layout transforms."""
    import ml_dtypes

    f16 = np.dtype(np.float16)
    f8 = np.dtype(ml_dtypes.float8_e4m3)

    x = np.asarray(x, np.float32)
    mask = np.asarray(mask)
    W_kv = np.asarray(W_kv, np.float32)
    b_kv = np.asarray(b_kv, np.float32)
    query = np.asarray(query, np.float32)
    W1 = np.asarray(W1, np.float32)
    b1 = np.asarray(b1, np.float32)
    W2 = np.asarray(W2, np.float32)
    b2 = np.asarray(b2, np.float32)

    W_k = W_kv[:, :E]
    W_v = W_kv[:, E:]
    # fold the per-head query into the k-projection: [E, H]
    w_att = np.einsum("ehd,hd->eh", W_k.reshape(E, H, D), query).astype(np.float32)

    addmask = np.where(mask[:, :, 0], np.float32(-1e30), np.float32(0.0))  # [B, N]

    watt_c = np.ascontiguousarray(w_att.astype(f16))
    wv_c = np.ascontiguousarray(W_v.astype(f16))
    w1_c = np.ascontiguousarray(W1.astype(f16))
    w2_c = np.ascontiguousarray(W2.astype(f16))
    bvT = np.ascontiguousarray(b_kv[E:].reshape(8, 128).T.astype(np.float32))
    b1T = np.ascontiguousarray(b1.reshape(32, 128).T.astype(np.float32))
    b2T = np.ascontiguousarray(b2.reshape(8, 128).T.astype(np.float32))

    in_maps = []
    for c in range(NCORES):
        in_maps.append(
            {
                "xT8": np.ascontiguousarray(x[c].T.astype(f8)),
                "x16": np.ascontiguousarray(x[c].astype(f16)),
                "watt": watt_c,
                # maskn[p, t] = addmask[n = 128*t + p]
                "maskn": np.ascontiguousarray(
                    addmask[c].reshape(32, 128).T.astype(np.float32)
                ),
                "wv": wv_c,
                "bvT": bvT,
                "w1": w1_c,
                "b1T": b1T,
                "w2": w2_c,
                "b2T": b2T,
            }
        )
    return in_maps


def kernel(**inputs):
    from concourse.bass_utils import run_bass_kernel_spmd

    in_maps = build_in_maps(**inputs)
    nc = get_nc()
    res = run_bass_kernel_spmd(nc, in_maps, list(range(NCORES)), trace=False)
    return np.stack([res.results[c]["out"][0] for c in range(NCORES)]).astype(
        np.float32
    )


# revision 9
# speedup vs baseline: 1.8892x; 1.1453x over previous
"""Trainium2 Bass kernel for nn_AttentionToVec (B=8, N=4096, E=1024, H=16, D=64).

Strategy: pure data-parallel over batch (1 batch element per NeuronCore), NO
collectives.  Each core computes its own row's full MLP with the complete
W1/W2 (profiling showed the AllGather/ReduceScatter + cc-barrier of the
tensor-parallel MLP cost ~100us, far more than the extra weight traffic).

DMA discipline: all loads go through the single HWDGE sync queue, which
drains FIFO in issue order.  Every stream tensor is fully resident in SBUF
(no pool-buffer gating), so the issue order IS the arrival order:
  xT (fp8, 4.2MB) -> x (fp16, 8.4MB) -> Wv (fp16, 2.1MB) -> W1 (fp8, 4.2MB)
  -> W2 (fp16, 8.4MB, sliced so the W2 matmuls pipeline behind arrival).
The W2 buffer reuses the xT pool's SBUF (xT pool closes after phase A).

Dtypes (validated vs reference on host, rel-err ~8.9e-3 vs 2e-2 budget):
  xT fp8e4m3 (mixed with fp16 stationary w_att), W1 fp8e4m3 (mixed with fp16
  moving s), everything else fp16; all matmul accumulation fp32 in PSUM.

Algebra (host does weight-only folding):
  - att logits = x @ w_att,  w_att[e,h] = sum_d W_k[e, h*D+d] * query[h,d]
    (the k-projection bias cancels inside softmax over n).
  - y[h,:] = sum_n exp_att[n,h] * x[n,:]  (deferred 1/Z normalization)
  - sampled[e] = (y[h(e),:] @ W_v[:, e]) + b_v[e],  h(e)=e//D.  Phase C
    computes ONLY the needed diagonal blocks, directly transposed:
    sfT_j[m, i] = sf[2j+i, 128j+m] so s[128j+m] = sfT_j[m, m//64].
  - MLP per-core on its own row, hidden laid out as zT[p, q] = z[128q+p]
    so gelu runs across all 128 partitions.
"""

import numpy as np

B = 8
N = 4096
E = 1024
H = 16
D = 64
HID = 4096
NCORES = 8

_CACHE = {}


def _build():
    import concourse.bacc as bacc
    import concourse.mybir as mybir
    from concourse import tile
    from concourse.masks import make_identity

    f32 = mybir.dt.float32
    f16 = mybir.dt.float16
    f8 = mybir.dt.float8e4
    Act = mybir.ActivationFunctionType
    Alu = mybir.AluOpType

    nc = bacc.Bacc(None, target_bir_lowering=False, debug=True, num_devices=NCORES)

    xT8 = nc.dram_tensor("xT8", [E, N], f8, kind="ExternalInput")
    x16 = nc.dram_tensor("x16", [N, E], f16, kind="ExternalInput")
    watt = nc.dram_tensor("watt", [E, H], f16, kind="ExternalInput")
    # packed [128, 80] f32: cols 0:32 maskn, 32:40 bvT, 40:72 b1T, 72:80 b2T
    cpack = nc.dram_tensor("cpack", [128, 80], f32, kind="ExternalInput")
    wv = nc.dram_tensor("wv", [E, E], f16, kind="ExternalInput")
    w1 = nc.dram_tensor("w1", [E, HID], f8, kind="ExternalInput")
    w2 = nc.dram_tensor("w2", [HID, E], f16, kind="ExternalInput")
    out = nc.dram_tensor("out", [1, E], f32, kind="ExternalOutput")

    with tile.TileContext(nc) as tc:
        with (
            tc.tile_pool(name="consts", bufs=1) as consts,
            tc.tile_pool(name="xp", bufs=1) as xp,
            tc.tile_pool(name="wvp", bufs=1) as wvp,
            tc.tile_pool(name="w1p", bufs=1) as w1p,
            tc.tile_pool(name="work", bufs=1) as work,
        ):
            identity = consts.tile([H, H], f16)
            make_identity(nc, identity[:])
            ones_s = consts.tile([128, 1], f16)
            nc.vector.memset(ones_s[:], 1.0)

            # ---- all DMA triggers in FIFO priority order ----
            watt_s = consts.tile([128, 8, H], f16)
            nc.sync.dma_start(
                out=watt_s[:], in_=watt.ap().rearrange("(c p) h -> p c h", p=128)
            )
            cp_s = consts.tile([128, 80], f32)
            nc.sync.dma_start(out=cp_s[:], in_=cpack[:, :])

            xtp_cm = tc.tile_pool(name="xtp", bufs=1)
            xtp = xtp_cm.__enter__()
            xT_s = xtp.tile([128, 8, N], f8)
            xTr = xT8.ap().rearrange("(g c p) n -> g p c n", g=4, p=128)
            for g in range(4):
                nc.sync.dma_start(out=xT_s[:, 2 * g : 2 * (g + 1), :], in_=xTr[g])

            x_s = xp.tile([128, 32, E], f16)
            xr = x16.ap().rearrange("(g r p) e -> g p r e", g=4, p=128)
            for g in range(4):
                nc.sync.dma_start(out=x_s[:, 8 * g : 8 * (g + 1), :], in_=xr[g])

            wv_s = wvp.tile([128, 8, 8, 128], f16)
            nc.sync.dma_start(
                out=wv_s[:],
                in_=wv.ap().rearrange("(c p) (j m) -> p c j m", p=128, m=128),
            )
            w1_s = w1p.tile([128, 8, 32, 128], f8)
            w1r = w1.ap().rearrange("(c p) (g q m) -> g p c q m", p=128, g=2, m=128)
            for g in range(2):
                nc.sync.dma_start(out=w1_s[:, :, 16 * g : 16 * (g + 1), :], in_=w1r[g])

            # ---- Phase A: attT[16, N] = w_att^T @ x^T (accumulate over e) ----
            psA_cm = tc.tile_pool(name="psA", bufs=1, space="PSUM")
            psA = psA_cm.__enter__()
            attT = psA.tile([H, N], f32)
            for c in range(8):
                for j in range(8):
                    nc.tensor.matmul(
                        attT[:, 512 * j : 512 * (j + 1)],
                        watt_s[:, c, :],
                        xT_s[:, c, 512 * j : 512 * (j + 1)],
                        start=(c == 0),
                        stop=(c == 7),
                    )

            # logits -> SBUF fp16 (mask is applied later as the exp bias)
            attm = work.tile([H, N], f16)
            for j in range(8):
                sl = slice(512 * j, 512 * (j + 1))
                nc.vector.tensor_copy(attm[:, sl], attT[:, sl])
            psA_cm.__exit__(None, None, None)
            xtp_cm.__exit__(None, None, None)

            # W2 buffer reuses xT's SBUF; its transfers queue after W1 (FIFO)
            w2p_cm = tc.tile_pool(name="w2p", bufs=1)
            w2p = w2p_cm.__enter__()
            w2_s = w2p.tile([128, 32, 8, 128], f16)
            w2r = w2.ap().rearrange("(q p) (g r m) -> g p q r m", p=128, g=4, m=128)
            for g in range(4):
                nc.sync.dma_start(out=w2_s[:, :, 2 * g : 2 * (g + 1), :], in_=w2r[g])

            psTr_cm = tc.tile_pool(name="psTr", bufs=4, space="PSUM")
            psTr = psTr_cm.__enter__()
            psB_cm = tc.tile_pool(name="psB", bufs=1, space="PSUM")
            psB = psB_cm.__enter__()

            # ---- Phase A2: per n-tile transpose + exp(logit + mask) -> fp16 ----
            att_n = work.tile([128, 32 * H], f16)
            for t in range(32):
                tr = psTr.tile([128, H], f16, tag="tr")
                nc.tensor.transpose(
                    tr[:], attm[:, 128 * t : 128 * (t + 1)], identity[:, :]
                )
                nc.scalar.activation(
                    att_n[:, H * t : H * (t + 1)],
                    tr[:],
                    Act.Exp,
                    bias=cp_s[:, t : t + 1],
                )

            # ---- Phase B: y[h,:] = sum_n p[n,h] x[n,:],  z[h] = sum_n p[n,h] ----
            y_ps = psB.tile([H, E], f32, tag="acc")
            z_ps = psB.tile([H, 1], f32, tag="accz")
            for t in range(32):
                lhs = att_n[:, H * t : H * (t + 1)]
                nc.tensor.matmul(
                    y_ps[:, 0:512],
                    lhs,
                    x_s[:, t, 0:512],
                    start=(t == 0),
                    stop=(t == 31),
                )
                nc.tensor.matmul(
                    y_ps[:, 512:1024],
                    lhs,
                    x_s[:, t, 512:1024],
                    start=(t == 0),
                    stop=(t == 31),
                )
                nc.tensor.matmul(
                    z_ps[:],
                    lhs,
                    ones_s[:],
                    start=(t == 0),
                    stop=(t == 31),
                )

            # normalize: y = y / z
            rz = work.tile([H, 1], f32)
            nc.vector.reciprocal(rz[:], z_ps[:, 0:1])
            y_s = work.tile([H, E], f16)
            nc.vector.tensor_scalar_mul(y_s[:], y_ps[:], rz[:])
            psB_cm.__exit__(None, None, None)

            # yT[e, h] chunks (fp16) for phase C
            yT = work.tile([128, 8 * H], f16)
            for j in range(8):
                tr2 = psTr.tile([128, H], f16, tag="tr")
                nc.tensor.transpose(
                    tr2[:], y_s[:, 128 * j : 128 * (j + 1)], identity[:, :]
                )
                nc.vector.tensor_copy(yT[:, H * j : H * (j + 1)], tr2[:])

            # ---- Phase C: diagonal blocks of sf = y @ Wv, directly transposed.
            # sfT_j[m, i] = sf[2j+i, 128j+m]; s[128j+m] = sfT_j[m, m//64].
            psC_cm = tc.tile_pool(name="psC", bufs=2, space="PSUM")
            psC = psC_cm.__enter__()
            s_f = work.tile([128, 8], f32)
            for j in range(8):
                sfT = psC.tile([128, 2], f32, tag="sf")
                for c in range(8):
                    nc.tensor.matmul(
                        sfT[:],
                        wv_s[:, c, j, :],
                        yT[:, 16 * c + 2 * j : 16 * c + 2 * j + 2],
                        start=(c == 0),
                        stop=(c == 7),
                    )
                nc.vector.tensor_copy(s_f[0:64, j : j + 1], sfT[0:64, 0:1])
                nc.vector.tensor_copy(s_f[64:128, j : j + 1], sfT[64:128, 1:2])
            psC_cm.__exit__(None, None, None)

            nc.vector.tensor_add(s_f[:], s_f[:], cp_s[:, 32:40])
            s16 = work.tile([128, 8], f16)
            nc.vector.tensor_copy(s16[:], s_f[:])

            # ---- Phase E: full MLP for this core's row ----
            psM_cm = tc.tile_pool(name="psM", bufs=1, space="PSUM")
            psM = psM_cm.__enter__()
            zT_ps = psM.tile([128, 32], f32, tag="z")
            for q in range(32):
                for c in range(8):
                    nc.tensor.matmul(
                        zT_ps[:, q : q + 1],
                        w1_s[:, c, q, :],
                        s16[:, c : c + 1],
                        start=(c == 0),
                        stop=(c == 7),
                    )

            # gelu (tanh approx, matches jax.nn.gelu default)
            z_s = work.tile([128, 32], f32, tag="zs")
            nc.vector.tensor_add(z_s[:], zT_ps[:], cp_s[:, 40:72])
            sq = work.tile([128, 32], f32, tag="ga")
            nc.scalar.activation(sq[:], z_s[:], Act.Square)
            cube = work.tile([128, 32], f32, tag="gb")
            nc.vector.tensor_mul(cube[:], sq[:], z_s[:])
            uu = work.tile([128, 32], f32, tag="ga")
            nc.vector.scalar_tensor_tensor(
                uu[:], cube[:], 0.044715, z_s[:], Alu.mult, Alu.add
            )
            th = work.tile([128, 32], f32, tag="gb")
            nc.scalar.activation(th[:], uu[:], Act.Tanh, scale=0.7978845608028654)
            hh = work.tile([128, 32], f32, tag="ga")
            nc.vector.scalar_tensor_tensor(
                hh[:], th[:], 1.0, z_s[:], Alu.add, Alu.mult
            )
            h16 = work.tile([128, 32], f16, tag="h16")
            nc.vector.tensor_scalar_mul(h16[:], hh[:], 0.5)

            oT_ps = psM.tile([128, 8], f32, tag="o")
            for r in range(8):
                for q in range(32):
                    nc.tensor.matmul(
                        oT_ps[:, r : r + 1],
                        w2_s[:, q, r, :],
                        h16[:, q : q + 1],
                        start=(q == 0),
                        stop=(q == 31),
                    )

            of = work.tile([128, 8], f32, tag="of")
            nc.vector.tensor_add(of[:], oT_ps[:], cp_s[:, 72:80])
            nc.vector.tensor_add(of[:], of[:], s_f[:])
            nc.sync.dma_start(
                out=out.ap().rearrange("o (j p) -> p (o j)", p=128), in_=of[:]
            )
            psM_cm.__exit__(None, None, None)
            psTr_cm.__exit__(None, None, None)
            w2p_cm.__exit__(None, None, None)

    return nc


def get_nc():
    if "nc" not in _CACHE:
        nc = _build()
        nc.finalize()
        _CACHE["nc"] = nc
    return _CACHE["nc"]


def build_in_maps(x, mask, W_kv, b_kv, query, W1, b1, W2, b2):
    """Host-side shard prep. Weight-only algebra + layout transforms."""
    import ml_dtypes

    f16 = np.dtype(np.float16)
    f8 = np.dtype(ml_dtypes.float8_e4m3)

    x = np.asarray(x, np.float32)
    mask = np.asarray(mask)
    W_kv = np.asarray(W_kv, np.float32)
    b_kv = np.asarray(b_kv, np.float32)
    query = np.asarray(query, np.float32)
    W1 = np.asarray(W1, np.float32)
    b1 = np.asarray(b1, np.float32)
    W2 = np.asarray(W2, np.float32)
    b2 = np.asarray(b2, np.float32)

    W_k = W_kv[:, :E]
    W_v = W_kv[:, E:]
    # fold the per-head query into the k-projection: [E, H]
    w_att = np.einsum("ehd,hd->eh", W_k.reshape(E, H, D), query).astype(np.float32)

    addmask = np.where(mask[:, :, 0], np.float32(-1e30), np.float32(0.0))  # [B, N]

    watt_c = np.ascontiguousarray(w_att.astype(f16))
    wv_c = np.ascontiguousarray(W_v.astype(f16))
    w1_c = np.ascontiguousarray(W1.astype(f8))
    w2_c = np.ascontiguousarray(W2.astype(f16))

    cpack_base = np.zeros((128, 80), np.float32)
    cpack_base[:, 32:40] = b_kv[E:].reshape(8, 128).T
    cpack_base[:, 40:72] = b1.reshape(32, 128).T
    cpack_base[:, 72:80] = b2.reshape(8, 128).T

    in_maps = []
    for c in range(NCORES):
        cp = cpack_base.copy()
        # maskn[p, t] = addmask[n = 128*t + p]
        cp[:, 0:32] = addmask[c].reshape(32, 128).T
        in_maps.append(
            {
                "xT8": np.ascontiguousarray(x[c].T.astype(f8)),
                "x16": np.ascontiguousarray(x[c].astype(f16)),
                "watt": watt_c,
                "cpack": cp,
                "wv": wv_c,
                "w1": w1_c,
                "w2": w2_c,
            }
        )
    return in_maps


def kernel(**inputs):
    from concourse.bass_utils import run_bass_kernel_spmd

    in_maps = build_in_maps(**inputs)
    nc = get_nc()
    res = run_bass_kernel_spmd(nc, in_maps, list(range(NCORES)), trace=False)
    return np.stack([res.results[c]["out"][0] for c in range(NCORES)]).astype(
        np.float32
    )


# revision 16
# speedup vs baseline: 2.3469x; 1.2423x over previous
"""Trainium2 Bass kernel for nn_AttentionToVec (B=8, N=4096, E=1024, H=16, D=64).

Strategy: pure data-parallel over batch (1 batch element per NeuronCore), NO
collectives.  Each core computes its own row's full MLP with the complete
W1/W2 (profiling showed the AllGather/ReduceScatter + cc-barrier of the
tensor-parallel MLP cost ~100us, far more than the extra weight traffic).

DMA discipline: all loads go through the single HWDGE sync queue, which
drains FIFO in issue order.  Every stream tensor is fully resident in SBUF
(no pool-buffer gating), so the issue order IS the arrival order:
  xT (fp8, 4.2MB) -> x (fp16, 8.4MB) -> Wv (fp16, 2.1MB) -> W1 (fp8, 4.2MB)
  -> W2 (fp16, 8.4MB, sliced so the W2 matmuls pipeline behind arrival).
The W2 buffer reuses the xT pool's SBUF (xT pool closes after phase A).

Dtypes (validated vs reference on host, rel-err ~8.9e-3 vs 2e-2 budget):
  xT fp8e4m3 (mixed with fp16 stationary w_att), W1 fp8e4m3 (mixed with fp16
  moving s), everything else fp16; all matmul accumulation fp32 in PSUM.

Algebra (host does weight-only folding):
  - att logits = x @ w_att,  w_att[e,h] = sum_d W_k[e, h*D+d] * query[h,d]
    (the k-projection bias cancels inside softmax over n).
  - y[h,:] = sum_n exp_att[n,h] * x[n,:]  (deferred 1/Z normalization)
  - sampled[e] = (y[h(e),:] @ W_v[:, e]) + b_v[e],  h(e)=e//D.  Phase C
    computes ONLY the needed diagonal blocks, directly transposed:
    sfT_j[m, i] = sf[2j+i, 128j+m] so s[128j+m] = sfT_j[m, m//64].
  - MLP per-core on its own row, hidden laid out as zT[p, q] = z[128q+p]
    so gelu runs across all 128 partitions.
"""

import numpy as np

B = 8
N = 4096
E = 1024
H = 16
D = 64
HID = 4096
NCORES = 8

_CACHE = {}


def _build():
    import concourse.bacc as bacc
    import concourse.mybir as mybir
    from concourse import tile
    from concourse.masks import make_identity

    f32 = mybir.dt.float32
    f16 = mybir.dt.float16
    f8 = mybir.dt.float8e4
    Act = mybir.ActivationFunctionType
    Alu = mybir.AluOpType

    nc = bacc.Bacc(None, target_bir_lowering=False, debug=True, num_devices=NCORES)

    xT8 = nc.dram_tensor("xT8", [E, N], f8, kind="ExternalInput")
    x16 = nc.dram_tensor("x16", [N, E], f16, kind="ExternalInput")
    watt = nc.dram_tensor("watt", [E, H], f16, kind="ExternalInput")
    # packed [128, 80] f32: cols 0:32 maskn, 32:40 bvT, 40:72 b1T, 72:80 b2T
    cpack = nc.dram_tensor("cpack", [128, 80], f32, kind="ExternalInput")
    wv = nc.dram_tensor("wv", [E, E], f16, kind="ExternalInput")
    w1 = nc.dram_tensor("w1", [E, HID], f8, kind="ExternalInput")
    w2 = nc.dram_tensor("w2", [HID, E], f16, kind="ExternalInput")
    # outT[p, j] = out_row[128*j + p]; host reassembles
    out = nc.dram_tensor("out", [128, 8], f32, kind="ExternalOutput")

    with tile.TileContext(nc) as tc:
        with (
            tc.tile_pool(name="consts", bufs=1) as consts,
            tc.tile_pool(name="xp", bufs=1) as xp,
            tc.tile_pool(name="wvp", bufs=1) as wvp,
            tc.tile_pool(name="w1p", bufs=1) as w1p,
            tc.tile_pool(name="work", bufs=1) as work,
        ):
            identity = consts.tile([H, H], f16)
            make_identity(nc, identity[:])
            ones_s = consts.tile([128, 1], f16)
            nc.vector.memset(ones_s[:], 1.0)

            # ---- all DMA triggers in FIFO priority order ----
            xtp_cm = tc.tile_pool(name="xtp", bufs=1)
            xtp = xtp_cm.__enter__()
            xT_s = xtp.tile([128, 8, N], f8)
            xTr = xT8.ap().rearrange("(g c p) n -> g p c n", g=4, p=128)
            nc.sync.dma_start(out=xT_s[:, 0:2, :], in_=xTr[0])

            watt_s = consts.tile([128, 8, H], f16)
            nc.sync.dma_start(
                out=watt_s[:], in_=watt.ap().rearrange("(c p) h -> p c h", p=128)
            )
            cp_s = consts.tile([128, 80], f32)
            nc.sync.dma_start(out=cp_s[:], in_=cpack[:, :])

            for g in range(1, 4):
                nc.sync.dma_start(out=xT_s[:, 2 * g : 2 * (g + 1), :], in_=xTr[g])

            x_s = xp.tile([128, 32, E], f16)
            xr = x16.ap().rearrange("(g r p) e -> g p r e", g=4, p=128)
            for g in range(4):
                nc.sync.dma_start(out=x_s[:, 8 * g : 8 * (g + 1), :], in_=xr[g])

            wv_s = wvp.tile([128, 8, 8, 128], f16)
            nc.sync.dma_start(
                out=wv_s[:],
                in_=wv.ap().rearrange("(c p) (j m) -> p c j m", p=128, m=128),
            )
            w1_s = w1p.tile([128, 8, 32, 128], f8)
            w1r = w1.ap().rearrange("(c p) (g q m) -> g p c q m", p=128, g=2, m=128)
            for g in range(2):
                nc.sync.dma_start(out=w1_s[:, :, 16 * g : 16 * (g + 1), :], in_=w1r[g])

            # ---- Phase A: attT[16, N] = w_att^T @ x^T (accumulate over e) ----
            psA_cm = tc.tile_pool(name="psA", bufs=1, space="PSUM")
            psA = psA_cm.__enter__()
            attT = psA.tile([H, N], f32)
            for c in range(8):
                for j in range(8):
                    nc.tensor.matmul(
                        attT[:, 512 * j : 512 * (j + 1)],
                        watt_s[:, c, :],
                        xT_s[:, c, 512 * j : 512 * (j + 1)],
                        start=(c == 0),
                        stop=(c == 7),
                    )

            # logits -> SBUF fp16 (mask is applied later as the exp bias)
            attm = work.tile([H, N], f16)
            for j in range(8):
                sl = slice(512 * j, 512 * (j + 1))
                nc.vector.tensor_copy(attm[:, sl], attT[:, sl])
            psA_cm.__exit__(None, None, None)
            xtp_cm.__exit__(None, None, None)

            # W2 buffer reuses xT's SBUF; its transfers queue after W1 (FIFO)
            w2p_cm = tc.tile_pool(name="w2p", bufs=1)
            w2p = w2p_cm.__enter__()
            w2_s = w2p.tile([128, 32, 8, 128], f16)
            w2r = w2.ap().rearrange("(g q p) (r m) -> g p q r m", g=4, p=128, m=128)
            for g in range(4):
                nc.sync.dma_start(out=w2_s[:, 8 * g : 8 * (g + 1), :, :], in_=w2r[g])

            psTr_cm = tc.tile_pool(name="psTr", bufs=4, space="PSUM")
            psTr = psTr_cm.__enter__()
            psB_cm = tc.tile_pool(name="psB", bufs=1, space="PSUM")
            psB = psB_cm.__enter__()

            # ---- Phase A2: per n-tile transpose + exp(logit + mask) -> fp16 ----
            att_n = work.tile([128, 32 * H], f16)
            for t in range(32):
                tr = psTr.tile([128, H], f16, tag="tr")
                nc.tensor.transpose(
                    tr[:], attm[:, 128 * t : 128 * (t + 1)], identity[:, :]
                )
                nc.scalar.activation(
                    att_n[:, H * t : H * (t + 1)],
                    tr[:],
                    Act.Exp,
                    bias=cp_s[:, t : t + 1],
                )

            # ---- Phase B: y[h,:] = sum_n p[n,h] x[n,:],  z[h] = sum_n p[n,h] ----
            y_ps = psB.tile([H, E], f32, tag="acc")
            z_ps = psB.tile([H, 1], f32, tag="accz")
            for t in range(32):
                lhs = att_n[:, H * t : H * (t + 1)]
                nc.tensor.matmul(
                    y_ps[:, 0:512],
                    lhs,
                    x_s[:, t, 0:512],
                    start=(t == 0),
                    stop=(t == 31),
                )
                nc.tensor.matmul(
                    y_ps[:, 512:1024],
                    lhs,
                    x_s[:, t, 512:1024],
                    start=(t == 0),
                    stop=(t == 31),
                )
                nc.tensor.matmul(
                    z_ps[:],
                    lhs,
                    ones_s[:],
                    start=(t == 0),
                    stop=(t == 31),
                )

            # normalize: y = y / z
            rz = work.tile([H, 1], f32)
            nc.vector.reciprocal(rz[:], z_ps[:, 0:1])
            y_s = work.tile([H, E], f16)
            nc.vector.tensor_scalar_mul(y_s[:], y_ps[:], rz[:])
            psB_cm.__exit__(None, None, None)

            # yT[e, h] chunks (fp16) for phase C
            yT = work.tile([128, 8 * H], f16)
            for j in range(8):
                tr2 = psTr.tile([128, H], f16, tag="tr")
                nc.tensor.transpose(
                    tr2[:], y_s[:, 128 * j : 128 * (j + 1)], identity[:, :]
                )
                nc.vector.tensor_copy(yT[:, H * j : H * (j + 1)], tr2[:])

            # ---- Phase C: diagonal blocks of sf = y @ Wv, directly transposed.
            # sfT_j[m, i] = sf[2j+i, 128j+m]; s[128j+m] = sfT_j[m, m//64].
            psC_cm = tc.tile_pool(name="psC", bufs=2, space="PSUM")
            psC = psC_cm.__enter__()
            s_f = work.tile([128, 8], f32)
            for j in range(8):
                sfT = psC.tile([128, 2], f32, tag="sf")
                for c in range(8):
                    nc.tensor.matmul(
                        sfT[:],
                        wv_s[:, c, j, :],
                        yT[:, 16 * c + 2 * j : 16 * c + 2 * j + 2],
                        start=(c == 0),
                        stop=(c == 7),
                    )
                nc.vector.tensor_copy(s_f[0:64, j : j + 1], sfT[0:64, 0:1])
                nc.vector.tensor_copy(s_f[64:128, j : j + 1], sfT[64:128, 1:2])
            psC_cm.__exit__(None, None, None)

            nc.vector.tensor_add(s_f[:], s_f[:], cp_s[:, 32:40])
            s16 = work.tile([128, 8], f16)
            nc.vector.tensor_copy(s16[:], s_f[:])

            # ---- Phase E: full MLP for this core's row ----
            psM_cm = tc.tile_pool(name="psM", bufs=1, space="PSUM")
            psM = psM_cm.__enter__()
            zT_ps = psM.tile([128, 32], f32, tag="z")
            for q in range(32):
                for c in range(8):
                    nc.tensor.matmul(
                        zT_ps[:, q : q + 1],
                        w1_s[:, c, q, :],
                        s16[:, c : c + 1],
                        start=(c == 0),
                        stop=(c == 7),
                    )

            # gelu (tanh approx, matches jax.nn.gelu default)
            z_s = work.tile([128, 32], f32, tag="zs")
            nc.vector.tensor_add(z_s[:], zT_ps[:], cp_s[:, 40:72])
            sq = work.tile([128, 32], f32, tag="ga")
            nc.scalar.activation(sq[:], z_s[:], Act.Square)
            cube = work.tile([128, 32], f32, tag="gb")
            nc.vector.tensor_mul(cube[:], sq[:], z_s[:])
            uu = work.tile([128, 32], f32, tag="ga")
            nc.vector.scalar_tensor_tensor(
                uu[:], cube[:], 0.044715, z_s[:], Alu.mult, Alu.add
            )
            th = work.tile([128, 32], f32, tag="gb")
            nc.scalar.activation(th[:], uu[:], Act.Tanh, scale=0.7978845608028654)
            hh = work.tile([128, 32], f32, tag="ga")
            nc.vector.scalar_tensor_tensor(
                hh[:], th[:], 1.0, z_s[:], Alu.add, Alu.mult
            )
            h16 = work.tile([128, 32], f16, tag="h16")
            nc.vector.tensor_scalar_mul(h16[:], hh[:], 0.5)

            # oT chains partial-accumulate per q-group so they pipeline
            # behind the 4 sliced w2 DMAs.  A start flag marks the whole 2KB
            # zero-region pending-zero, so only the very first matmul of the
            # tile may carry it; later first-touches of other columns still
            # overwrite via the lazy pending-zero bytes.
            oT_ps = psM.tile([128, 8], f32, tag="o")
            for g in range(4):
                for r in range(8):
                    for q in range(8 * g, 8 * (g + 1)):
                        nc.tensor.matmul(
                            oT_ps[:, r : r + 1],
                            w2_s[:, q, r, :],
                            h16[:, q : q + 1],
                            start=(g == 0 and r == 0 and q == 0),
                            stop=(g == 3 and r == 7 and q == 31),
                            skip_group_check=True,
                        )

            of = work.tile([128, 8], f32, tag="of")
            nc.vector.tensor_add(of[:], oT_ps[:], cp_s[:, 72:80])
            nc.vector.tensor_add(of[:], of[:], s_f[:])
            nc.sync.dma_start(out=out[:, :], in_=of[:])
            psM_cm.__exit__(None, None, None)
            psTr_cm.__exit__(None, None, None)
            w2p_cm.__exit__(None, None, None)

    return nc


def get_nc():
    if "nc" not in _CACHE:
        nc = _build()
        nc.finalize()
        _CACHE["nc"] = nc
    return _CACHE["nc"]


def build_in_maps(x, mask, W_kv, b_kv, query, W1, b1, W2, b2):
    """Host-side shard prep. Weight-only algebra + layout transforms."""
    import ml_dtypes

    f16 = np.dtype(np.float16)
    f8 = np.dtype(ml_dtypes.float8_e4m3)

    x = np.asarray(x, np.float32)
    mask = np.asarray(mask)
    W_kv = np.asarray(W_kv, np.float32)
    b_kv = np.asarray(b_kv, np.float32)
    query = np.asarray(query, np.float32)
    W1 = np.asarray(W1, np.float32)
    b1 = np.asarray(b1, np.float32)
    W2 = np.asarray(W2, np.float32)
    b2 = np.asarray(b2, np.float32)

    W_k = W_kv[:, :E]
    W_v = W_kv[:, E:]
    # fold the per-head query into the k-projection: [E, H]
    w_att = np.einsum("ehd,hd->eh", W_k.reshape(E, H, D), query).astype(np.float32)

    addmask = np.where(mask[:, :, 0], np.float32(-1e30), np.float32(0.0))  # [B, N]

    watt_c = np.ascontiguousarray(w_att.astype(f16))
    wv_c = np.ascontiguousarray(W_v.astype(f16))
    w1_c = np.ascontiguousarray(W1.astype(f8))
    w2_c = np.ascontiguousarray(W2.astype(f16))

    cpack_base = np.zeros((128, 80), np.float32)
    cpack_base[:, 32:40] = b_kv[E:].reshape(8, 128).T
    cpack_base[:, 40:72] = b1.reshape(32, 128).T
    cpack_base[:, 72:80] = b2.reshape(8, 128).T

    in_maps = []
    for c in range(NCORES):
        cp = cpack_base.copy()
        # maskn[p, t] = addmask[n = 128*t + p]
        cp[:, 0:32] = addmask[c].reshape(32, 128).T
        in_maps.append(
            {
                "xT8": np.ascontiguousarray(x[c].T.astype(f8)),
                "x16": np.ascontiguousarray(x[c].astype(f16)),
                "watt": watt_c,
                "cpack": cp,
                "wv": wv_c,
                "w1": w1_c,
                "w2": w2_c,
            }
        )
    return in_maps


def kernel(**inputs):
    from concourse.bass_utils import run_bass_kernel_spmd

    in_maps = build_in_maps(**inputs)
    nc = get_nc()
    res = run_bass_kernel_spmd(nc, in_maps, list(range(NCORES)), trace=False)
    # out is [128, 8] with out_row[128*j + p] = out[p, j]
    return np.stack(
        [np.asarray(res.results[c]["out"]).T.reshape(-1) for c in range(NCORES)]
    ).astype(np.float32)
